# revision 34
# baseline (speedup 1.0000x reference)
"""Two-layer GAT forward on 8 Trainium2 NeuronCores — v2.

Layer 1 does NOT gather: the host pre-gathers x[src] per (dst-sharded,
dst-sorted, padded) edge slot and ships it transposed in fp8; the device
runs a per-chunk DoubleRow fp8 edge-GEMM (x[src] @ W1aug) straight into
PSUM, applies the attention weight p during the PSUM eviction (split
between Vector and Scalar engines), and scatter-adds with shipped fp8
one-hot matrices via DoubleRow matmuls over chunk pairs.  The h1->h2
GEMM is fused per window; an AllGather shares h2; layer 2 gathers 288B
h2 rows per edge (padded slots idx=-1, skipped) and aggregates the same
way with a single head.

Self-contained: hardcodes the problem shapes from the spec.
"""
import os
import sys
import numpy as np

try:
    import concourse.bass as bass  # noqa
except ImportError:
    sys.path.insert(0, "/opt/trn_rl_repo")

import concourse.bass as bass
import concourse.tile as tile
from concourse import mybir, bacc
from concourse.bass_utils import run_bass_kernel_spmd

# ---------------------------------------------------------------- problem dims
N, E = 10000, 160000
FIN, H1, C1, C2 = 256, 8, 128, 128
D1 = H1 * C1  # 1024
SLOPE = 0.2
NDEV = 8
NLOC = N // NDEV  # 1250
NW = (NLOC + 127) // 128  # 10 windows of 128 dsts (last = 98)
LASTW = NLOC - (NW - 1) * 128  # 98

GCOL = D1 + 8  # edge-GEMM cols: h 1024 | s_src 8
ROW2 = 256  # layer-2 gather row (bf16, 512B, %256 rule)
ROWC = 136  # compact AllGather row (bf16): h2 128 | s2src 1 | pad
R2K = 132  # layer-2 scaled row (fp8): h2*p 128 | p 1 | pad

F32 = mybir.dt.float32
BF16 = mybir.dt.bfloat16
FP8 = mybir.dt.float8e4
I16 = mybir.dt.int16
ALU = mybir.AluOpType
ACTF = mybir.ActivationFunctionType
DR = mybir.MatmulPerfMode.DoubleRow

_EPS = 1e-30
EVICT_DVE_OF16 = int(os.environ.get("KDVE16", "4"))
KDEFER = int(os.environ.get("KDEFER", "2"))  # chunks (c%16)<K -> DVE

# --------------------------------------------------------------------- patches


def _apply_drain_patch():
    """This walrus build rejects >1 sync-wait on the Tile-exit Drain; split the
    waits across consecutive drains (semantically identical)."""
    from concourse.vector_clock import ScopedClock

    def _patched(self, tick_clock, wait_clock):
        drain_inst = self.nc.sync.drain()
        wait_clock.add_sem_waits(
            drain_inst.ins, ScopedClock({None: tick_clock.global_clock})
        )
        si = drain_inst.ins.sync_info
        if si is not None and len(si.on_wait) > 1:
            waits = list(si.on_wait)
            si.on_wait = waits[:1]
            drain_inst.ins.sync_info = si
            for i in range(1, len(waits)):
                extra = self.nc.sync.drain()
                esi = extra.ins.sync_info
                if esi is None:
                    esi = mybir.SyncInfo(on_wait=[], on_update=[])
                esi.on_wait = list(esi.on_wait) + waits[i : i + 1]
                extra.ins.sync_info = esi
        self.nc.all_engine_barrier()
        assert self.sems is not None
        popped = self.nc._tile_sem_poison_stack.pop()
        assert popped is self._sem_poison
        self.nc.clear_and_free_semaphores(list(self.sems.allocated().values()))
        self.nc.all_engine_barrier()

    tile.TileContext._drain_and_barrier = _patched


_apply_drain_patch()

# ------------------------------------------------------------------- host prep


def _bf(a):
    import ml_dtypes

    return np.ascontiguousarray(a).astype(ml_dtypes.bfloat16)


def _f8(a):
    import ml_dtypes

    return np.ascontiguousarray(np.clip(a, -240.0, 240.0)).astype(
        ml_dtypes.float8_e4m3fn
    )


def _wrap2(a, k):
    """[k*128, M] -> [128, k*M] with inner [k, M] layout."""
    m = a.shape[1]
    return np.ascontiguousarray(
        a.reshape(k, 128, m).transpose(1, 0, 2).reshape(128, k * m)
    )


def _wrap_idx(idx):
    """dma_gather index layout: idx i at partition i%16, col i//16, replicated
    8x across the 128 partitions."""
    a = np.ascontiguousarray(idx.astype(np.int16).reshape(-1, 16).T)
    return np.ascontiguousarray(np.tile(a, (8, 1)))


def _prep_edges(edge_index):
    """Shard self-loop-augmented edges by dst across devices, sort by dst.
    Window w gets CW[w] chunks of 128 edge slots (even count for DoubleRow
    pairs, max over devices).  Padded slots: src=0 (for the x pre-gather),
    gather idx=-1 (skipped), dst one-hot all-zero."""
    loops = np.arange(N, dtype=np.int64)
    src = np.concatenate([np.asarray(edge_index[0], np.int64), loops])
    dst = np.concatenate([np.asarray(edge_index[1], np.int64), loops])

    per_dev = []
    cnts_all = np.zeros((NDEV, NW), np.int64)
    for d in range(NDEV):
        base = d * NLOC
        sel = (dst >= base) & (dst < base + NLOC)
        s_d, t_d = src[sel], dst[sel]
        order = np.argsort(t_d, kind="stable")
        s_d, t_d = s_d[order], t_d[order]
        wid = (t_d - base) // 128
        cnts_all[d] = np.bincount(wid, minlength=NW)
        per_dev.append((s_d, t_d))

    cw = ((cnts_all.max(axis=0) + 127) // 128).astype(np.int64)
    cw = ((cw + 1) // 2) * 2  # even chunk count per window (DR pairs)
    off = np.concatenate([[0], np.cumsum(cw)])
    nchunk = int(off[-1])
    tot = nchunk * 128

    devs = []
    for d in range(NDEV):
        base = d * NLOC
        s_d, t_d = per_dev[d]
        cnts = cnts_all[d]
        srcp = np.zeros(tot, np.int64)
        gidx = np.full(tot, int(os.environ.get("KPADIDX", "0")), np.int64)
        dloc = np.full(tot, -1.0, np.float32)
        starts = np.concatenate([[0], np.cumsum(cnts)])
        for w in range(NW):
            a, b = starts[w], starts[w + 1]
            o = int(off[w]) * 128
            n = int(b - a)
            srcp[o : o + n] = s_d[a:b]
            gidx[o : o + n] = s_d[a:b]
            dloc[o : o + n] = (t_d[a:b] - base - w * 128).astype(np.float32)
        # one-hots: oh[c, e, dstcol]
        dl = dloc.reshape(nchunk, 128)
        oh = dl[:, :, None] == np.arange(128, dtype=np.float32)
        stg = oh.transpose(1, 0, 2).reshape(128, nchunk * 128)  # [e, c*128+d]
        ssb = oh.transpose(2, 0, 1).reshape(128, nchunk * 128)  # [d, c*128+e]
        devs.append(
            {
                "srcp": srcp,
                "stg": _bf(stg),
                "ssb": _bf(ssb),
                "srcidx": _wrap_idx(gidx),
            }
        )
    return devs, cw, off, nchunk, tot


# -------------------------------------------------------------- program build

_CACHE = {}


def _build(cw, off, nchunk, tot, add_b1, add_b2):
    phases = os.environ.get("KPH", "AB")
    cwmax = int(max(cw))
    nc = bacc.Bacc(None, dynamic_dma_scratch_size=81920, num_swdge_queues=2)
    dp = nc.declare_dram_parameter
    # shared inputs
    W1f8_d = dp("W1f8", [128, 2 * GCOL], BF16, isOutput=False)
    W1ad_d = dp("W1ad", [128, 2 * 8], BF16, isOutput=False)
    W2_d = dp("W2aug", [128, 8 * 130], BF16, isOutput=False)
    if add_b1:
        b1_d = dp("b1bc", [128, D1], F32, isOutput=False)
    if add_b2:
        b2_d = dp("b2bc", [128, C2], F32, isOutput=False)
    # per-device inputs
    xgT_d = dp("xgT", [128, 2 * tot], BF16, isOutput=False)
    xlocT_d = dp("xlocT", [128, 2 * NLOC], BF16, isOutput=False)
    stg_d = dp("stg", [128, nchunk * 128], BF16, isOutput=False)
    ssb_d = dp("ssb", [128, nchunk * 128], BF16, isOutput=False)
    srcidx_d = dp("srcidx", [128, tot // 16], I16, isOutput=False)
    # output
    out_d = dp("out", [NLOC, C2], F32, isOutput=True)
    # internal DRAM
    h2loc = nc.dram_tensor("h2loc", [NLOC, ROWC], BF16)
    h2allC = nc.dram_tensor("h2allC", [N, ROWC], BF16, addr_space="Shared")
    h2all = nc.dram_tensor("h2all", [N, ROW2], BF16)

    with tile.TileContext(nc) as tc:
        with tc.tile_pool(name="const", bufs=1) as constp:
            W1f8_t = constp.tile([128, 2, GCOL], BF16)
            nc.sync.dma_start(
                W1f8_t[:], W1f8_d[:].rearrange("p (k n) -> p k n", k=2)
            )
            W1ad_t = constp.tile([128, 2, 8], BF16)
            nc.sync.dma_start(
                W1ad_t[:], W1ad_d[:].rearrange("p (k n) -> p k n", k=2)
            )
            W2_t = constp.tile([128, 8, 130], BF16)
            nc.sync.dma_start(W2_t[:], W2_d[:].rearrange("p (k n) -> p k n", k=8))
            xloc_t = constp.tile([128, 2, NLOC], BF16)
            nc.sync.dma_start(
                xloc_t[:], xlocT_d[:].rearrange("p (k n) -> p k n", k=2)
            )
            srcidx_t = constp.tile([128, tot // 16], I16)
            nc.sync.dma_start(srcidx_t[:], srcidx_d[:])
            if add_b1:
                b1_t = constp.tile([128, D1], F32)
                nc.sync.dma_start(b1_t[:], b1_d[:])
            if add_b2:
                b2_t = constp.tile([128, C2], F32)
                nc.sync.dma_start(b2_t[:], b2_d[:])
            h1T_t = constp.tile([128, 8, NW * 128], BF16)
            s2d_t = constp.tile([128, NW], BF16)
            sdw_all = constp.tile([128, NW * 8], BF16)

            # ---------------- Layer 1 (+ fused h2 GEMM per window)
            with (
                tc.tile_pool(name="xg", bufs=2) as xgp,
                tc.tile_pool(name="st", bufs=2) as stp,
                tc.tile_pool(name="sb", bufs=2) as sbp,
                tc.tile_pool(name="hg", bufs=4) as hgp,
                tc.tile_pool(name="sc", bufs=3) as scp,
                tc.tile_pool(name="we", bufs=2) as wep,
                tc.tile_pool(name="psE", bufs=5, space="PSUM") as psE_p,
                tc.tile_pool(name="psw", bufs=1, space="PSUM") as psw_p,
                tc.tile_pool(name="aux", bufs=1, space="PSUM") as aux_p,
            ):
                # upfront: window dst scores sdw[d, h] = x[dst] @ W1@a_dst
                auxs = aux_p.tile([128, cwmax * 8 + 8], F32, tag="aux")
                for w in range(NW):
                    wl = 128 if w < NW - 1 else LASTW
                    for k in range(2):
                        nc.tensor.matmul(
                            auxs[:wl, w * 8 : (w + 1) * 8],
                            xloc_t[:, k, w * 128 : w * 128 + wl],
                            W1ad_t[:, k, :],
                            start=(k == 0),
                            stop=(k == 1),
                        )
                nc.vector.memset(sdw_all[:], 0.0)
                for w in range(NW):
                    wl = 128 if w < NW - 1 else LASTW
                    nc.vector.tensor_copy(
                        sdw_all[:wl, w * 8 : (w + 1) * 8],
                        auxs[:wl, w * 8 : (w + 1) * 8],
                    )
                for w in range(NW):
                    wl = 128 if w < NW - 1 else LASTW
                    o = int(off[w])
                    ncw = int(cw[w])
                    # stream this window's inputs
                    xg_t = xgp.tile([128, 2, cwmax * 128], BF16, tag="xg")
                    nc.sync.dma_start(
                        xg_t[:, :, 0 : ncw * 128],
                        xgT_d[:]
                        .rearrange("p (k n) -> p k n", k=2)[
                            :, :, o * 128 : (o + ncw) * 128
                        ],
                    )
                    st_t = stp.tile([128, cwmax, 128], BF16, tag="st")
                    nc.sync.dma_start(
                        st_t[:, 0:ncw, :],
                        stg_d[:, o * 128 : (o + ncw) * 128].rearrange(
                            "p (c d) -> p c d", d=128
                        ),
                    )
                    sb_t = sbp.tile([128, cwmax * 128], BF16, tag="sb")
                    nc.sync.dma_start(
                        sb_t[:, 0 : ncw * 128],
                        ssb_d[:, o * 128 : (o + ncw) * 128],
                    )
                    aux_t = aux_p.tile([128, cwmax * 8 + 8], F32, tag="aux")
                    # score pass: s_src (GEMM cols) + s_dst (esp) for all chunks
                    for c in range(ncw):
                        for k in range(2):
                            nc.tensor.matmul(
                                aux_t[:, c * 8 : (c + 1) * 8],
                                xg_t[:, k, c * 128 : (c + 1) * 128],
                                W1f8_t[:, k, 1024:1032],
                                start=(k == 0),
                                stop=False,
                                skip_group_check=True,
                            )
                        nc.tensor.matmul(
                            aux_t[:, c * 8 : (c + 1) * 8],
                            sb_t[:, c * 128 : (c + 1) * 128],
                            sdw_all[:, w * 8 : (w + 1) * 8],
                            start=False,
                            stop=True,
                            skip_group_check=True,
                        )
                    # p = exp(lrelu(s)) = max(exp(s), exp(0.2 s)), batched
                    e1 = scp.tile([128, cwmax * 8], F32, tag="e1")
                    nc.scalar.activation(
                        e1[:, 0 : ncw * 8], aux_t[:, 0 : ncw * 8], ACTF.Exp
                    )
                    e2 = scp.tile([128, cwmax * 8], F32, tag="e2")
                    nc.scalar.activation(
                        e2[:, 0 : ncw * 8], aux_t[:, 0 : ncw * 8], ACTF.Exp,
                        scale=SLOPE,
                    )
                    pbf = scp.tile([128, cwmax * 8], F32, tag="pbf")
                    nc.vector.tensor_tensor(
                        pbf[:, 0 : ncw * 8], e1[:, 0 : ncw * 8],
                        e2[:, 0 : ncw * 8], ALU.max,
                    )
                    pbw = scp.tile([128, cwmax * 8], BF16, tag="pbw")
                    nc.vector.tensor_copy(pbw[:, 0 : ncw * 8], pbf[:, 0 : ncw * 8])

                    psw = psw_p.tile([128, D1], F32, tag="psw")
                    den = aux_t[:, cwmax * 8 : cwmax * 8 + 8]

                    def _scatter(c):
                        lhsC = st_t[:, c, :]
                        first = c == 0
                        last = c == ncw - 1
                        for hf in range(2):
                            nc.tensor.matmul(
                                psw[:, hf * 512 : hf * 512 + 512], lhsC,
                                hgq[c][:, hf * 512 : hf * 512 + 512],
                                start=first, stop=last,
                            )
                        nc.tensor.matmul(
                            den, lhsC, pbw[:, c * 8 : (c + 1) * 8],
                            start=first, stop=last, skip_group_check=True,
                        )

                    hgq = {}
                    for c in range(ncw):
                        psh = []
                        for hf in range(2):
                            ps = psE_p.tile([128, 512], F32, tag="psE")
                            psh.append(ps)
                            for k in range(2):
                                nc.tensor.matmul(
                                    ps[:],
                                    xg_t[:, k, c * 128 : (c + 1) * 128],
                                    W1f8_t[:, k, hf * 512 : hf * 512 + 512],
                                    start=(k == 0),
                                    stop=(k == 1),
                                )
                        # evict + p-scale
                        hg_t = hgp.tile([128, D1], BF16, tag="hg")
                        for hf in range(2):
                            nc.vector.tensor_tensor(
                                hg_t[:, hf * 512 : hf * 512 + 512].rearrange(
                                    "e (h c) -> e h c", c=C1
                                ),
                                psh[hf][:].rearrange("e (h c) -> e h c", c=C1),
                                pbf[:, c * 8 + hf * 4 : c * 8 + hf * 4 + 4]
                                .unsqueeze(2)
                                .broadcast_to([128, 4, C1]),
                                ALU.mult,
                            )
                        hgq[c] = hg_t
                        if c >= KDEFER:
                            _scatter(c - KDEFER)
                    for cc in range(ncw - KDEFER, ncw):
                        _scatter(cc)
                    # ---- window epilogue: h1 = elu(psw/den + b1)
                    dens = wep.tile([128, 8], F32, tag="dens")
                    nc.vector.tensor_scalar(dens[:], den, _EPS, None, ALU.max)
                    rec = wep.tile([128, 8], F32, tag="rec")
                    nc.vector.reciprocal(rec[:], dens[:])
                    h1r = wep.tile([128, D1], F32, tag="h1r")
                    for h in range(8):
                        nc.scalar.activation(
                            h1r[:, h * 128 : (h + 1) * 128],
                            psw[:, h * 128 : (h + 1) * 128],
                            ACTF.Copy,
                            scale=rec[:, h : h + 1],
                        )
                    if add_b1:
                        nc.vector.tensor_tensor(h1r[:], h1r[:], b1_t[:], ALU.add)
                    mn = wep.tile([128, D1], F32, tag="mn")
                    nc.vector.tensor_scalar(mn[:], h1r[:], 0.0, None, ALU.min)
                    em = wep.tile([128, D1], F32, tag="em")
                    nc.scalar.activation(em[:], mn[:], ACTF.Exp)
                    rl = wep.tile([128, D1], F32, tag="rl")
                    nc.scalar.activation(rl[:], h1r[:], ACTF.Relu)
                    sm = wep.tile([128, D1], F32, tag="sm")
                    nc.vector.tensor_tensor(sm[:], rl[:], em[:], ALU.add)
                    h1b = wep.tile([128, D1], BF16, tag="h1b")
                    nc.vector.tensor_scalar(h1b[:], sm[:], -1.0, None, ALU.add)
                    nc.sync.dma_start_transpose(
                        h1T_t[:, :, w * 128 : w * 128 + 128], h1b[:]
                    )
                    # ---- fused h2 = h1 @ W2aug for this window
                    ps2 = psE_p.tile([128, 512], F32, tag="psE")
                    for k in range(8):
                        nc.tensor.matmul(
                            ps2[:wl, 0:130],
                            h1T_t[:, k, w * 128 : w * 128 + wl],
                            W2_t[:, k, :],
                            start=(k == 0),
                            stop=(k == 7),
                        )
                    h2t = wep.tile([128, ROWC], BF16, tag="h2t")
                    nc.vector.tensor_copy(h2t[:wl, 0:129], ps2[:wl, 0:129])
                    nc.vector.tensor_copy(
                        s2d_t[:wl, w : w + 1], ps2[:wl, 129:130]
                    )
                    nc.sync.dma_start(
                        h2loc[w * 128 : w * 128 + wl, :], h2t[:wl, :]
                    )

            if "B" not in phases and "G" not in phases:
                nc.gpsimd.dma_start(out_d[:, :], h2loc[:, 0:C2])

            if "G" in phases and "B" not in phases:
                nc.gpsimd.collective_compute(
                    "AllGather",
                    ALU.bypass,
                    ins=[h2loc[:]],
                    outs=[h2allC[:]],
                    replica_groups=[list(range(NDEV))],
                )
                nc.sync.dma_start(h2all[:, 0:ROWC], h2allC[:, :])
                nc.gpsimd.dma_start(out_d[:, :], h2allC[0:NLOC, 0:C2])

            # ---------------- Layer 2
            if "B" in phases:
              with (
                tc.tile_pool(name="g2", bufs=2) as g2p,
                tc.tile_pool(name="gh", bufs=2) as ghp,
                tc.tile_pool(name="st2", bufs=2) as stp2,
                tc.tile_pool(name="sb2", bufs=2) as sbp2,
                tc.tile_pool(name="sc2", bufs=3) as scp2,
                tc.tile_pool(name="we2", bufs=2) as wep2,
                tc.tile_pool(name="ps2", bufs=1, space="PSUM") as ps2_p,
                tc.tile_pool(name="ax2", bufs=2, space="PSUM") as ax2_p,
            ):
                gstep = int(os.environ.get("KGSTEP", "8"))
                # prep window-0 gather descriptors during L1; trigger post-AG
                kprep = os.environ.get("KPREP", "0") == "1"
                dma_sem = nc.alloc_semaphore("l2g0_dma")
                g2_t0 = g2p.tile([128, cwmax, ROW2], BF16, tag="g2")
                ncw0 = int(cw[0])
                for c0 in range(0, ncw0, gstep) if kprep else []:
                    c1 = min(ncw0, c0 + gstep)
                    nc.gpsimd.dma_gather(
                        out_ap=g2_t0[:, c0:c1, :],
                        in_ap=h2all[:, :],
                        idxs_ap=srcidx_t[:, c0 * 8 : c1 * 8],
                        num_idxs=(c1 - c0) * 128,
                        num_idxs_reg=(c1 - c0) * 128,
                        elem_size=ROW2,
                        single_packet=True,
                        prepare_only=True,
                        sem=dma_sem,
                        queue_num=1,
                    )
                nc.gpsimd.collective_compute(
                    "AllGather",
                    ALU.bypass,
                    ins=[h2loc[:]],
                    outs=[h2allC[:]],
                    replica_groups=[list(range(NDEV))],
                )
                nc.sync.dma_start(h2all[:, 0:ROWC], h2allC[:, :])
                if kprep:
                    nc.gpsimd.trigger_dma(count=None, queue_num=1)
                for w in range(NW):
                    wl = 128 if w < NW - 1 else LASTW
                    o = int(off[w])
                    ncw = int(cw[w])
                    st_t = stp2.tile([128, cwmax, 128], BF16, tag="st")
                    nc.sync.dma_start(
                        st_t[:, 0:ncw, :],
                        stg_d[:, o * 128 : (o + ncw) * 128].rearrange(
                            "p (c d) -> p c d", d=128
                        ),
                    )
                    sb_t = sbp2.tile([128, cwmax * 128], BF16, tag="sb")
                    nc.sync.dma_start(
                        sb_t[:, 0 : ncw * 128],
                        ssb_d[:, o * 128 : (o + ncw) * 128],
                    )
                    if w == 0:
                        g2_t = g2_t0
                    else:
                        g2_t = g2p.tile([128, cwmax, ROW2], BF16, tag="g2")
                    for c0 in (range(0, ncw, gstep) if (w > 0 or not kprep) else []):
                        c1 = min(ncw, c0 + gstep)
                        nc.gpsimd.dma_gather(
                            out_ap=g2_t[:, c0:c1, :],
                            in_ap=h2all[:, :],
                            idxs_ap=srcidx_t[:, (o + c0) * 8 : (o + c1) * 8],
                            num_idxs=(c1 - c0) * 128,
                            num_idxs_reg=(c1 - c0) * 128,
                            elem_size=ROW2,
                            single_packet=os.environ.get("KSP", "1") == "1",
                        )
                    # per-edge dst score
                    aux2 = ax2_p.tile([128, cwmax], F32, tag="ax")
                    for c in range(ncw):
                        nc.tensor.matmul(
                            aux2[:, c : c + 1],
                            sb_t[:, c * 128 : (c + 1) * 128],
                            s2d_t[:, w : w + 1],
                            start=True, stop=True,
                        )
                    pts = scp2.tile([128, cwmax], F32, tag="pts")
                    nc.vector.tensor_tensor(
                        pts[:, 0:ncw], aux2[:, 0:ncw], g2_t[:, 0:ncw, 128],
                        ALU.add,
                    )
                    e1 = scp2.tile([128, cwmax], F32, tag="e1")
                    nc.scalar.activation(e1[:, 0:ncw], pts[:, 0:ncw], ACTF.Exp)
                    e2 = scp2.tile([128, cwmax], F32, tag="e2")
                    nc.scalar.activation(
                        e2[:, 0:ncw], pts[:, 0:ncw], ACTF.Exp, scale=SLOPE
                    )
                    gh_t = ghp.tile([128, cwmax, R2K], BF16, tag="gh")
                    nc.vector.tensor_tensor(
                        gh_t[:, 0:ncw, 128], e1[:, 0:ncw], e2[:, 0:ncw], ALU.max
                    )
                    nc.vector.tensor_tensor(
                        gh_t[:, 0:ncw, 0:128],
                        g2_t[:, 0:ncw, 0:128],
                        gh_t[:, 0:ncw, 128]
                        .unsqueeze(2)
                        .broadcast_to([128, ncw, 128]),
                        ALU.mult,
                    )
                    psw2 = ps2_p.tile([128, 132], F32, tag="pw")
                    for c in range(ncw):
                        nc.tensor.matmul(
                            psw2[:, 0:129],
                            st_t[:, c, :],
                            gh_t[:, c, 0:129],
                            start=(c == 0),
                            stop=(c == ncw - 1),
                        )
                    dens = wep2.tile([128, 1], F32, tag="dens")
                    nc.vector.tensor_scalar(
                        dens[:], psw2[:, 128:129], _EPS, None, ALU.max
                    )
                    rec2 = wep2.tile([128, 1], F32, tag="rec")
                    nc.vector.reciprocal(rec2[:], dens[:])
                    ot = wep2.tile([128, C2], F32, tag="ot")
                    nc.vector.tensor_scalar(
                        ot[:], psw2[:, 0:128], rec2[:, 0:1], None, ALU.mult
                    )
                    if add_b2:
                        nc.vector.tensor_tensor(ot[:], ot[:], b2_t[:], ALU.add)
                    nc.sync.dma_start(
                        out_d[w * 128 : w * 128 + wl, :], ot[:wl, :]
                    )

    nc.finalize()
    return nc


# ------------------------------------------------------------------ entrypoint

TRACE = [False]
LAST = [None]


def kernel(x, edge_index, W1, a_src1, a_dst1, b1, W2, a_src2, a_dst2, b2):
    x = np.asarray(x, np.float32)
    W1 = np.asarray(W1, np.float32)
    W2 = np.asarray(W2, np.float32)
    a_src1 = np.asarray(a_src1, np.float32)
    a_dst1 = np.asarray(a_dst1, np.float32)
    a_src2 = np.asarray(a_src2, np.float32)
    a_dst2 = np.asarray(a_dst2, np.float32)
    b1 = np.asarray(b1, np.float32)
    b2 = np.asarray(b2, np.float32)
    ei = np.asarray(edge_index)

    devs, cw, off, nchunk, tot = _prep_edges(ei)

    W1aug = np.concatenate([W1, W1 @ _fold_a(a_src1)], axis=1)  # [256, 1032]
    W1ad = W1 @ _fold_a(a_dst1)  # [256, 8]
    W2aug = np.concatenate(
        [W2, W2 @ a_src2[0][:, None], W2 @ a_dst2[0][:, None]], axis=1
    )  # [1024, 130]
    add_b1 = bool(np.any(b1 != 0))
    add_b2 = bool(np.any(b2 != 0))

    key = (tuple(int(v) for v in cw), add_b1, add_b2)
    if key not in _CACHE:
        _CACHE[key] = _build(cw, off, nchunk, tot, add_b1, add_b2)
    nc = _CACHE[key]

    shared = {
        "W1f8": _bf(_wrap2(W1aug, 2)),
        "W1ad": _bf(_wrap2(W1ad, 2)),
        "W2aug": _bf(_wrap2(W2aug, 8)),
    }
    if add_b1:
        shared["b1bc"] = np.ascontiguousarray(np.tile(b1[None, :], (128, 1)))
    if add_b2:
        shared["b2bc"] = np.ascontiguousarray(np.tile(b2[None, :], (128, 1)))

    in_maps = []
    for d in range(NDEV):
        dv = devs[d]
        xg = x[dv["srcp"]]  # [tot, 256]
        base = d * NLOC
        in_maps.append(
            {
                **shared,
                "xgT": _bf(_wrap2(np.ascontiguousarray(xg.T), 2)),
                "xlocT": _bf(
                    _wrap2(np.ascontiguousarray(x[base : base + NLOC].T), 2)
                ),
                "stg": dv["stg"],
                "ssb": dv["ssb"],
                "srcidx": dv["srcidx"],
            }
        )
    res = run_bass_kernel_spmd(nc, in_maps, list(range(NDEV)), trace=TRACE[0])
    LAST[0] = res
    out = np.concatenate([res.results[d]["out"] for d in range(NDEV)], axis=0)
    return out.astype(np.float32)


def _fold_a(a):
    """[H, C] per-head attention vecs -> [D1, H] block-diagonal projector."""
    heads = a.shape[0]
    A = np.zeros((D1, heads), np.float32)
    for h in range(heads):
        A[h * C1 : (h + 1) * C1, h] = a[h]
    return A
